# revision 1
# baseline (speedup 1.0000x reference)
"""Supervised contrastive loss (nn_Batch_CL) on 8 Trainium2 NeuronCores.

Math (per the reference):
  x = l2_normalize(feature_embeds)            # [N, D]
  logits = (x @ x.T) / tau                    # tau = 0.1
  Z_i    = sum_{j != i} exp(logits[i, j])
  S_i    = sum_{j != i, l_j == l_i} logits[i, j]
  P_i    = |{j != i : l_j == l_i}|
  per_row_i = S_i / P_i - log Z_i   (if P_i > 0 else 0)
  loss = -sum(per_row) / n_valid

Distribution: rows sharded 8 ways (1024 rows/core). Each core receives the
full feature matrix with ITS OWN rows permuted to the front, so the diagonal
of its logits block lands at a statically-known position (cols m*128..+127 of
column-group 0 for row-chunk m) — no core-id branching; the SPMD program is
identical, only input data differs per core.

Per-core kernel strategy:
  - exp+row-sum fused in one ACT instruction per [128, 2048] PSUM block via
    activation(Exp, scale=10, accum_out=...): the Z reduction is free.
  - positive-pair sums via class aggregation: Msum = x_hat^T @ onehot(labels)
    accumulated on PE (borrowing a main-pool PSUM slot per group, drained to
    SBUF by a small DVE add), then F = x_hat_block @ Msum gives per-(row,
    class) sums; a one-hot mask + accum_out selects S_i. No NxN mask work.
  - exact diagonal terms extracted from the PSUM logits blocks with an
    identity-mask scalar_tensor_tensor + accum_out, so Z_i excludes e^{l_ii}
    bit-exactly and S_i excludes l_ii.
  - l2 normalization: rsqrt(s) = Exp(-0.5 * Ln(s)) on ACT — stays in the
    natural_log_exp table set used by the main exp (no table-set thrash).
  - x^T (contraction layout) built with batched bf16 DMA-xbar transposes
    (one [128, 8, 128] block-transpose instruction per 1024 columns).

Outputs per core: [sum of valid per_row over its 1024 rows, its n_valid].
Host epilogue: loss = -sum(parts) / sum(n_valid).
"""

import numpy as np

N = 8192
D = 128
N_CORES = 8
ROWS_PER_CORE = N // N_CORES          # 1024
NCHUNK = N // 128                     # 64 chunks of 128 rows
GROUPS = [1024, 2048, 2048, 2048, 1024]   # column group widths
NGROUP = len(GROUPS)
GW = 2048                             # max group width (psum tile size)
HALF = 1024                           # build granularity
CH = HALF // 128                      # chunks per half-build (8)
NOWN = ROWS_PER_CORE // 128           # 8 own row-chunks
NCLS = 33
INV_TAU = 10.0
DEBUG_OUTPUTS = False

_NC = None

# ---------------------------------------------------------------------------
# Inlined workarounds (kernel.py must be self-contained).
#
# The local walrus build accepts at most ONE sync-wait command per
# instruction (any type). Tile's scheduler attaches several. Two fixes:
#   1. TileContext._drain_and_barrier is replaced so the exit drain's many
#      waits are split across single-wait nops.
#   2. split_multiwait(nc): post-pass that hoists extra sync waits from any
#      instruction onto injected same-engine EventSemaphore instructions
#      placed immediately before it (engines are in-order, so this is
#      semantically identical).
# ---------------------------------------------------------------------------

_nop_counter = [0]


def _split_drain_and_barrier(self, tick_clock, wait_clock):
    import bass_rust

    vec = tick_clock.global_clock  # VectorClock
    for proc in range(len(vec)):
        tickv = vec[proc]
        if tickv > 0:
            nop_inst = self.nc.sync.nop(nofuse=True)
            c = bass_rust.ScopedClock()
            c.require_at_least(None, proc, tickv)
            wait_clock.add_sem_waits(nop_inst.ins, c)
    self.nc.sync.drain()
    self.nc.all_engine_barrier()
    assert self.sems is not None
    popped = self.nc._tile_sem_poison_stack.pop()
    assert popped is self._sem_poison
    self.nc.clear_and_free_semaphores(list(self.sems.allocated().values()))
    self.nc.all_engine_barrier()


def _install_tile_patch():
    from concourse import tile as _tile

    _tile.TileContext._drain_and_barrier = _split_drain_and_barrier


def _split_multiwait(nc):
    """Hoist all-but-one sync wait from every instruction onto nops."""
    import concourse.mybir as mybir

    n_hoisted = 0
    for bb in nc.main_func.blocks:
        insns = bb.instructions
        out = []
        changed = False
        for ins in insns:
            si = ins.sync_info
            if si is not None and len(si.on_wait) > 1:
                waits = list(si.on_wait)
                for w in waits[:-1]:
                    _nop_counter[0] += 1
                    nop = mybir.InstEventSemaphore(
                        name=f"hoistnop-{_nop_counter[0]}",
                        engine=ins.engine,
                        sync_info=mybir.SyncInfo(on_wait=[w], on_update=[]),
                    )
                    out.append(nop)
                    n_hoisted += 1
                ins.sync_info = mybir.SyncInfo(
                    on_wait=[waits[-1]], on_update=list(si.on_update)
                )
                changed = True
            out.append(ins)
        if changed:
            bb.instructions = out
    return n_hoisted


def _install_ntff_hook():
    """Synthesize the antenv.axon_hooks module missing from this image so
    run_bass_kernel_spmd(trace=True) can NTFF-profile under axon."""
    import sys
    import types

    if "antenv.axon_hooks" in sys.modules:
        return True
    try:
        import antenv
        from trn_agent_boot.trn_boot import _ntff_profile_via_ctypes
    except ImportError:
        return False
    hook_box = [None]
    mod = types.ModuleType("antenv.axon_hooks")
    mod.set_axon_ntff_profile_hook = lambda h: hook_box.__setitem__(0, h)
    mod.get_axon_ntff_profile_hook = lambda: hook_box[0]
    sys.modules["antenv.axon_hooks"] = mod
    antenv.axon_hooks = mod
    hook = _ntff_profile_via_ctypes("/opt/axon/libaxon_pjrt.so")
    mod.set_axon_ntff_profile_hook(hook)
    return hook is not None



def _build_nc(split_waits=True):
    import concourse.bass as bass
    import concourse.mybir as mybir
    from concourse import tile
    from contextlib import ExitStack

    _install_tile_patch()

    f32 = mybir.dt.float32
    bf16 = mybir.dt.bfloat16
    Alu = mybir.AluOpType
    Act = mybir.ActivationFunctionType
    X = mybir.AxisListType.X

    nc = bass.Bass()
    x_dram = nc.dram_tensor("xperm", [N, D], f32, kind="ExternalInput")
    lab_dram = nc.dram_tensor("labels_pc", [128, NCHUNK], f32, kind="ExternalInput")
    iota_dram = nc.dram_tensor("iota33", [128, NCLS], f32, kind="ExternalInput")
    eye33_dram = nc.dram_tensor("eye33", [NCLS, NCLS], f32, kind="ExternalInput")
    out_dram = nc.dram_tensor("out", [2], f32, kind="ExternalOutput")
    if DEBUG_OUTPUTS:
        dbg = {
            name: nc.dram_tensor(name, shape, f32, kind="ExternalOutput")
            for name, shape in [
                ("dbg_zpart", [128, NGROUP * NOWN]),
                ("dbg_rawdiag", [128, NOWN]),
                ("dbg_pown", [128, NOWN]),
                ("dbg_sfull", [128, NOWN]),
                ("dbg_parts", [128, 2]),
            ]
        }

    with tile.TileContext(nc) as tc, ExitStack() as ctx:
        persist = ctx.enter_context(tc.tile_pool(name="persist", bufs=1))

        xT = persist.tile([128, N], bf16)                 # normalized, transposed
        O_bf = persist.tile([128, NCHUNK * NCLS], bf16)   # one-hot labels (PE operand)
        O_own = persist.tile([128, NOWN * NCLS], f32)     # one-hot, own chunks (DVE)
        cnt_bcast = persist.tile([128, NCLS], f32)
        Zpart = persist.tile([128, NGROUP * NOWN], f32)
        rawdiag = persist.tile([128, NOWN], f32)
        P_own = persist.tile([128, NOWN], f32)
        S_full = persist.tile([128, NOWN], f32)
        Msum_sb = persist.tile([NCLS, 128], f32)          # summed class sums
        Msum_parts = persist.tile([NCLS, NGROUP * 128], f32)  # per-group partials
        labels_sb = persist.tile([128, NCHUNK], f32)
        iota_sb = persist.tile([128, NCLS], f32)
        eye33_sb = persist.tile([NCLS, NCLS], f32)
        ones_f = persist.tile([128, 1], f32)
        ones_row = persist.tile([1, 128], f32)
        cnt_row = persist.tile([1, NCLS], f32)
        Mt_sb = persist.tile([128, NCLS], bf16)
        dump33 = persist.tile([128, NCLS], f32)
        e_dump = persist.tile([128, GW], f32)             # ACT out scratch (unread)
        res_sb = persist.tile([1, 2], f32)

        Zrow = persist.tile([128, NOWN], f32)
        e_diag = persist.tile([128, NOWN], f32)
        Zexcl = persist.tile([128, NOWN], f32)
        lnZ = persist.tile([128, NOWN], f32)
        S_excl = persist.tile([128, NOWN], f32)
        P_pos = persist.tile([128, NOWN], f32)
        P_safe = persist.tile([128, NOWN], f32)
        P_inv = persist.tile([128, NOWN], f32)
        valid = persist.tile([128, NOWN], f32)
        t_sp = persist.tile([128, NOWN], f32)
        perrow = persist.tile([128, NOWN], f32)
        loss_parts = persist.tile([128, 2], f32)
        cnt_part = persist.tile([128, NCLS], f32)

        # ---------------- prologue ----------------
        nc.gpsimd.dma_start(labels_sb[:], lab_dram[:])
        nc.gpsimd.dma_start(iota_sb[:], iota_dram[:])
        nc.gpsimd.dma_start(eye33_sb[:], eye33_dram[:])
        nc.vector.memset(ones_f[:], 1.0)
        nc.vector.memset(ones_row[:], 1.0)
        # one-hot labels; DVE runs this while the first feature chunk streams
        nc.vector.tensor_tensor(
            out=O_bf[:].rearrange("p (c k) -> p c k", k=NCLS),
            in0=iota_sb[:].rearrange("p (a k) -> p a k", a=1)
            .to_broadcast((128, NCHUNK, NCLS)),
            in1=labels_sb[:].to_broadcast((128, NCHUNK, NCLS)),
            op=Alu.is_equal,
        )

        # ---------------- main: build + compute, group by group ----------------
        with (
            tc.tile_pool(name="main_ps", bufs=2, space="PSUM") as main_ps,
            tc.tile_pool(name="build", bufs=2) as build_pool,
        ):
            gstart = 0
            for g, gw in enumerate(GROUPS):
                nhalf = gw // HALF
                # --- build group g of xT: half-builds of 1024 cols ---
                xh_halves = []
                for h in range(nhalf):
                    base = gstart + h * HALF          # column offset
                    xs = build_pool.tile([128, HALF], f32, tag=f"xs{h}")
                    nc.sync.dma_start(
                        xs[:].rearrange("p (c d) -> p c d", d=128),
                        x_dram[base:base + HALF, :].rearrange(
                            "(c p) d -> p c d", p=128),
                    )
                    sq = build_pool.tile([128, HALF], f32, tag=f"sq{h}")
                    nc.vector.tensor_mul(sq[:], xs[:], xs[:])
                    ssq = build_pool.tile([128, CH], f32, tag=f"ssq{h}")
                    nc.vector.reduce_sum(
                        ssq[:], sq[:].rearrange("p (c d) -> p c d", d=128), axis=X)
                    lns = build_pool.tile([128, CH], f32, tag=f"lns{h}")
                    nc.scalar.activation(lns[:], ssq[:], Act.Ln)
                    rinv = build_pool.tile([128, CH], f32, tag=f"rinv{h}")
                    nc.scalar.activation(rinv[:], lns[:], Act.Exp, scale=-0.5)
                    xh = build_pool.tile([128, HALF], bf16, tag=f"xh{h}")
                    nc.vector.scalar_tensor_tensor(
                        out=xh[:].rearrange("p (c r) -> p c r", r=128),
                        in0=xs[:].rearrange("p (c r) -> p c r", r=128),
                        scalar=1.0,
                        in1=rinv[:].to_broadcast((128, CH, 128)),
                        op0=Alu.mult,
                        op1=Alu.mult,
                    )
                    nc.sync.dma_start_transpose(
                        xT[:, base:base + HALF].rearrange("p (c r) -> p c r", r=128),
                        xh[:],
                    )
                    if g == 0 and h == 0:
                        # diagonal terms: ||x_hat_bf16||^2 per own row, matching
                        # the bf16 products the PE matmul will accumulate
                        sq2 = build_pool.tile([128, HALF], f32, tag="sq2")
                        nc.vector.tensor_mul(sq2[:], xh[:], xh[:])
                        nc.vector.reduce_sum(
                            rawdiag[:],
                            sq2[:].rearrange("p (c d) -> p c d", d=128), axis=X)
                    xh_halves.append(xh)

                def emit_msum_block():
                    # class-sum accumulation: lhsT = one-hot chunk (33-col
                    # LDWEIGHTS), out = [33, 128]; borrows one main-pool slot
                    # briefly, then drains into SBUF
                    mps = main_ps.tile([128, GW], f32, tag="e", name="mps")
                    for h in range(nhalf):
                        for i in range(CH):
                            c = gstart // 128 + h * CH + i
                            nc.tensor.matmul(
                                mps[0:NCLS, 0:128],
                                O_bf[:, c * NCLS:(c + 1) * NCLS],
                                xh_halves[h][:, i * 128:(i + 1) * 128],
                                start=(h == 0 and i == 0),
                                stop=(h == nhalf - 1 and i == CH - 1),
                            )
                    nc.scalar.copy(
                        Msum_parts[:, g * 128:(g + 1) * 128], mps[0:NCLS, 0:128])

                # --- logits + exp + rowsum for all 8 own row-chunks ---
                for m in range(NOWN):
                    ps = main_ps.tile([128, GW], f32, tag="e")
                    lhsT = xT[:, m * 128:(m + 1) * 128]
                    for k in range(gw // 512):
                        nc.tensor.matmul(
                            ps[:, k * 512:(k + 1) * 512],
                            lhsT,
                            xT[:, gstart + k * 512: gstart + (k + 1) * 512],
                            start=True, stop=True,
                        )
                    nc.scalar.activation(
                        e_dump[:, 0:gw], ps[:, 0:gw], Act.Exp, scale=INV_TAU,
                        accum_out=Zpart[:, g * NOWN + m: g * NOWN + m + 1],
                    )
                    # slot the class-sum matmuls into the PE stream mid-loop
                    # (group 0: at the end, so the one-hot build has landed)
                    if m == (NOWN - 1 if g == 0 else 3):
                        emit_msum_block()

                if g == 2:
                    # per-row positive-count chain; runs in mid-main idle time
                    nc.vector.tensor_tensor(
                        out=O_own[:].rearrange("p (c k) -> p c k", k=NCLS),
                        in0=iota_sb[:].rearrange("p (a k) -> p a k", a=1)
                        .to_broadcast((128, NOWN, NCLS)),
                        in1=labels_sb[:, 0:NOWN].to_broadcast(
                            (128, NOWN, NCLS)),
                        op=Alu.is_equal,
                    )
                    nc.vector.reduce_sum(
                        cnt_part[:],
                        O_bf[:].rearrange("p (c k) -> p k c", k=NCLS), axis=X)
                gstart += gw

        # ---------------- epilogue ----------------
        with tc.tile_pool(name="epi_ps", bufs=1, space="PSUM") as epi_ps:
            cnt_ps = epi_ps.tile([1, NCLS], f32, tag="cnt")
            nc.tensor.matmul(cnt_ps[:], ones_f[:], cnt_part[:], start=True, stop=True)
            nc.vector.tensor_copy(cnt_row[:], cnt_ps[:])
            cntb_ps = epi_ps.tile([128, NCLS], f32, tag="cntb")
            nc.tensor.matmul(cntb_ps[:], ones_row[:], cnt_row[:], start=True, stop=True)
            nc.vector.tensor_copy(cnt_bcast[:], cntb_ps[:])
            for m in range(NOWN):
                nc.vector.scalar_tensor_tensor(
                    out=dump33[:],
                    in0=O_own[:, m * NCLS:(m + 1) * NCLS],
                    scalar=1.0,
                    in1=cnt_bcast[:],
                    op0=Alu.mult,
                    op1=Alu.mult,
                    accum_out=P_own[:, m:m + 1],
                )
            nc.vector.reduce_sum(
                Msum_sb[:],
                Msum_parts[:].rearrange("p (g d) -> p d g", g=NGROUP), axis=X)
            mt_ps = epi_ps.tile([128, NCLS], f32, tag="mt")
            nc.tensor.transpose(mt_ps[:], Msum_sb[:], eye33_sb[:])
            nc.vector.tensor_copy(Mt_sb[:], mt_ps[:])
            F_ps = epi_ps.tile([128, NOWN * NCLS], f32, tag="F")
            for m in range(NOWN):
                nc.tensor.matmul(
                    F_ps[:, m * NCLS:(m + 1) * NCLS],
                    xT[:, m * 128:(m + 1) * 128],
                    Mt_sb[:],
                    start=True, stop=True,
                )
            for m in range(NOWN):
                nc.vector.scalar_tensor_tensor(
                    out=dump33[:],
                    in0=F_ps[:, m * NCLS:(m + 1) * NCLS],
                    scalar=1.0,
                    in1=O_own[:, m * NCLS:(m + 1) * NCLS],
                    op0=Alu.mult,
                    op1=Alu.mult,
                    accum_out=S_full[:, m:m + 1],
                )

            nc.vector.reduce_sum(
                Zrow[:], Zpart[:].rearrange("p (g m) -> p m g", m=NOWN), axis=X)
            nc.scalar.activation(e_diag[:], rawdiag[:], Act.Exp, scale=INV_TAU)
            nc.vector.tensor_sub(Zexcl[:], Zrow[:], e_diag[:])
            nc.scalar.activation(lnZ[:], Zexcl[:], Act.Ln)

            nc.vector.tensor_sub(S_excl[:], S_full[:], rawdiag[:])
            nc.vector.tensor_scalar_add(P_pos[:], P_own[:], -1.0)
            nc.vector.tensor_scalar_max(P_safe[:], P_pos[:], 1.0)
            nc.vector.reciprocal(P_inv[:], P_safe[:])
            nc.vector.tensor_scalar_min(valid[:], P_pos[:], 1.0)  # P>=0 integer
            nc.vector.scalar_tensor_tensor(
                out=t_sp[:], in0=S_excl[:], scalar=INV_TAU, in1=P_inv[:],
                op0=Alu.mult, op1=Alu.mult,
            )
            nc.vector.tensor_sub(perrow[:], t_sp[:], lnZ[:])
            nc.vector.tensor_mul(perrow[:], perrow[:], valid[:])

            nc.vector.reduce_sum(loss_parts[:, 0:1], perrow[:], axis=X)
            nc.vector.reduce_sum(loss_parts[:, 1:2], valid[:], axis=X)
            sum_ps = epi_ps.tile([1, 2], f32, tag="sum")
            nc.tensor.matmul(sum_ps[:], ones_f[:], loss_parts[:], start=True, stop=True)
            nc.vector.tensor_copy(res_sb[:], sum_ps[:])
            nc.sync.dma_start(out_dram[:].rearrange("(a b) -> a b", a=1), res_sb[:])
            if DEBUG_OUTPUTS:
                nc.sync.dma_start(dbg["dbg_zpart"][:], Zpart[:])
                nc.sync.dma_start(dbg["dbg_rawdiag"][:], rawdiag[:])
                nc.sync.dma_start(dbg["dbg_pown"][:], P_own[:])
                nc.sync.dma_start(dbg["dbg_sfull"][:], S_full[:])
                nc.sync.dma_start(dbg["dbg_parts"][:], loss_parts[:])

    if split_waits:
        _split_multiwait(nc)
    return nc


def _get_nc(split_waits=True):
    global _NC
    if _NC is None:
        _NC = _build_nc(split_waits)
    return _NC


def _make_in_maps(x, lab):
    iota = np.ascontiguousarray(
        np.tile(np.arange(NCLS, dtype=np.float32), (128, 1))
    )
    in_maps = []
    for c in range(N_CORES):
        lo, hi = c * ROWS_PER_CORE, (c + 1) * ROWS_PER_CORE
        perm = np.concatenate(
            [np.arange(lo, hi), np.arange(0, lo), np.arange(hi, N)]
        )
        xp = np.ascontiguousarray(x[perm])
        lp = np.ascontiguousarray(
            lab[perm].astype(np.float32).reshape(NCHUNK, 128).T
        )
        in_maps.append(
            {"xperm": xp, "labels_pc": lp, "iota33": iota,
             "eye33": np.eye(NCLS, dtype=np.float32)}
        )
    return in_maps


def _combine(results):
    parts = np.stack([np.asarray(results[c]["out"]) for c in range(N_CORES)])
    loss = -parts[:, 0].sum() / parts[:, 1].sum()
    return np.array(loss, dtype=np.float32)


def kernel(feature_embeds, label_ids):
    from concourse.bass_utils import run_bass_kernel_spmd

    x = np.asarray(feature_embeds, dtype=np.float32)
    lab = np.asarray(label_ids)
    nc = _get_nc()
    res = run_bass_kernel_spmd(nc, _make_in_maps(x, lab), list(range(N_CORES)))
    return _combine(res.results)


def kernel_profiled(feature_embeds, label_ids):
    """Same as kernel(), but with NTFF tracing; returns (loss, exec_time_ns)."""
    print("ntff hook installed:", _install_ntff_hook())
    from concourse.bass_utils import run_bass_kernel_spmd

    x = np.asarray(feature_embeds, dtype=np.float32)
    lab = np.asarray(label_ids)
    nc = _get_nc()
    res = run_bass_kernel_spmd(
        nc, _make_in_maps(x, lab), list(range(N_CORES)), trace=True
    )
    return _combine(res.results), res.exec_time_ns



# revision 14
# speedup vs baseline: 1.1376x; 1.1376x over previous
"""Supervised contrastive loss (nn_Batch_CL) on 8 Trainium2 NeuronCores.

Math (per the reference):
  x = l2_normalize(feature_embeds)            # [N, D]
  logits = (x @ x.T) / tau                    # tau = 0.1
  Z_i    = sum_{j != i} exp(logits[i, j])
  S_i    = sum_{j != i, l_j == l_i} logits[i, j]
  P_i    = |{j != i : l_j == l_i}|
  per_row_i = S_i / P_i - log Z_i   (if P_i > 0 else 0)
  loss = -sum(per_row) / n_valid

Distribution (symmetric-halving, circulant bands): exp(L) is symmetric, so
each exp needs computing only once.  Global row-chunk i (of 64) computes the
band of column-chunks d = 0..32 (mod 64): 4224 columns.  Row-sums of a band
block cover Z for its rows; column-sums cover Z for its columns (the mirror
block is never computed).  d=32 blocks are computed twice fleet-wide, so
their exp carries bias=ln(1/2).  Core c owns row-chunks 8c..8c+7; its input
is x rotated by 1024c rows, making all band columns local indices
128m..128m+4223 (max 5119) -- the SPMD program is identical on every core.

Per-core kernel:
  - band logits via PE (bf16), exp+row-sum fused in ACT via accum_out.
  - column sums: per-128-col stationary-e matmuls (e_sub^T @ ones) into a
    per-chunk PSUM region -- each [128,1] output lands in partition layout.
  - positive-pair sums via class aggregation (Msum = x_hat^T @ onehot) as a
    single PSUM accumulation over all 64 chunks.
  - l2 normalization: squaring+reduce on GPSIMD (Pool), rsqrt = Exp(-.5 Ln)
    on ACT (stays in the natural_log_exp table set), scale on DVE.
Host epilogue assembles Z from the row/col partials (rolled by each core's
rotation), then loss = -sum(valid*(S/P/tau - ln Z)) / n_valid.
"""

import numpy as np

N = 8192
D = 128
N_CORES = 8
ROWS_PER_CORE = N // N_CORES          # 1024
NCHUNK = N // 128                     # 64 chunks of 128 rows
NOWN = 8                              # own row-chunks per core
NHALF = 8                             # 1024-row build halves
HALF = 1024
CH = HALF // 128                      # chunks per half (8)
NXT = 5                               # halves that need transposing (band cols)
XTW = 5120                            # xT width (max band col + 1)
BANDW = 4224                          # band width per chunk (d=0..32)
MAINW = 4096                          # band minus the d32 block
PIECES = ((0, 1536), (1536, 1536), (3072, 1024))  # ACT pieces of the main band
NCLS = 33
INV_TAU = 10.0
LNHALF = float(np.log(0.5))
DEBUG_OUTPUTS = False

_NC = None

# ---------------------------------------------------------------------------
# Inlined workarounds (kernel.py must be self-contained).
#
# The local walrus build accepts at most ONE sync-wait command per
# instruction (any type). Tile's scheduler attaches several. Two fixes:
#   1. TileContext._drain_and_barrier is replaced so the exit drain's many
#      waits are split across single-wait nops.
#   2. split_multiwait(nc): post-pass that hoists extra sync waits from any
#      instruction onto injected same-engine EventSemaphore instructions
#      placed immediately before it (engines are in-order, so this is
#      semantically identical).
# ---------------------------------------------------------------------------

_nop_counter = [0]


def _split_drain_and_barrier(self, tick_clock, wait_clock):
    import bass_rust

    vec = tick_clock.global_clock  # VectorClock
    for proc in range(len(vec)):
        tickv = vec[proc]
        if tickv > 0:
            nop_inst = self.nc.sync.nop(nofuse=True)
            c = bass_rust.ScopedClock()
            c.require_at_least(None, proc, tickv)
            wait_clock.add_sem_waits(nop_inst.ins, c)
    self.nc.sync.drain()
    self.nc.all_engine_barrier()
    assert self.sems is not None
    popped = self.nc._tile_sem_poison_stack.pop()
    assert popped is self._sem_poison
    self.nc.clear_and_free_semaphores(list(self.sems.allocated().values()))
    self.nc.all_engine_barrier()


def _install_tile_patch():
    from concourse import tile as _tile

    _tile.TileContext._drain_and_barrier = _split_drain_and_barrier


def _split_multiwait(nc):
    """Hoist all-but-one sync wait from every instruction onto nops."""
    import concourse.mybir as mybir

    n_hoisted = 0
    for bb in nc.main_func.blocks:
        insns = bb.instructions
        out = []
        changed = False
        for ins in insns:
            si = ins.sync_info
            if si is not None and len(si.on_wait) > 1:
                waits = list(si.on_wait)
                for w in waits[:-1]:
                    _nop_counter[0] += 1
                    nop = mybir.InstEventSemaphore(
                        name=f"hoistnop-{_nop_counter[0]}",
                        engine=ins.engine,
                        sync_info=mybir.SyncInfo(on_wait=[w], on_update=[]),
                    )
                    out.append(nop)
                    n_hoisted += 1
                ins.sync_info = mybir.SyncInfo(
                    on_wait=[waits[-1]], on_update=list(si.on_update)
                )
                changed = True
            out.append(ins)
        if changed:
            bb.instructions = out
    return n_hoisted


def _install_ntff_hook():
    """Synthesize the antenv.axon_hooks module missing from this image so
    run_bass_kernel_spmd(trace=True) can NTFF-profile under axon."""
    import sys
    import types

    if "antenv.axon_hooks" in sys.modules:
        return True
    try:
        import antenv
        from trn_agent_boot.trn_boot import _ntff_profile_via_ctypes
    except ImportError:
        return False
    hook_box = [None]
    mod = types.ModuleType("antenv.axon_hooks")
    mod.set_axon_ntff_profile_hook = lambda h: hook_box.__setitem__(0, h)
    mod.get_axon_ntff_profile_hook = lambda: hook_box[0]
    sys.modules["antenv.axon_hooks"] = mod
    antenv.axon_hooks = mod
    hook = _ntff_profile_via_ctypes("/opt/axon/libaxon_pjrt.so")
    mod.set_axon_ntff_profile_hook(hook)
    return hook is not None


def _build_nc(split_waits=True):
    import concourse.bass as bass
    import concourse.mybir as mybir
    from concourse import tile
    from contextlib import ExitStack

    _install_tile_patch()

    f32 = mybir.dt.float32
    bf16 = mybir.dt.bfloat16
    Alu = mybir.AluOpType
    Act = mybir.ActivationFunctionType
    X = mybir.AxisListType.X

    nc = bass.Bass()
    x_dram = nc.dram_tensor("xperm", [N, D], f32, kind="ExternalInput")
    lab_dram = nc.dram_tensor("labels_pc", [128, NCHUNK], f32, kind="ExternalInput")
    iota_dram = nc.dram_tensor("iota33", [128, NCLS], f32, kind="ExternalInput")
    eye33_dram = nc.dram_tensor("eye33", [NCLS, NCLS], f32, kind="ExternalInput")
    zrow_dram = nc.dram_tensor("zrow", [128, NOWN], f32, kind="ExternalOutput")
    zcol_dram = nc.dram_tensor("zcol", [128, NOWN * 32], f32, kind="ExternalOutput")
    tsp_dram = nc.dram_tensor("tsp", [128, NOWN], f32, kind="ExternalOutput")
    valid_dram = nc.dram_tensor("valid", [128, NOWN], f32, kind="ExternalOutput")
    if DEBUG_OUTPUTS:
        dbg_msum = nc.dram_tensor("dbg_msum", [NCLS, 128], f32, kind="ExternalOutput")
        dbg_sfull = nc.dram_tensor("dbg_sfull", [128, NOWN], f32, kind="ExternalOutput")
        dbg_pown = nc.dram_tensor("dbg_pown", [128, NOWN], f32, kind="ExternalOutput")
        dbg_raw = nc.dram_tensor("dbg_raw", [128, NOWN], f32, kind="ExternalOutput")

    with tile.TileContext(nc) as tc, ExitStack() as ctx:
        persist = ctx.enter_context(tc.tile_pool(name="persist", bufs=1))

        xT = persist.tile([128, XTW], bf16)               # normalized, transposed
        e_d32 = persist.tile([128, NOWN * 128], bf16)     # exp of d32 blocks (x0.5)
        O_bf = persist.tile([128, NCHUNK * NCLS], bf16)   # one-hot labels (PE operand)
        O_own = persist.tile([128, NOWN * NCLS], f32)     # one-hot, own chunks (DVE)
        Zacc = persist.tile([128, 3 * NOWN], f32)         # exp accum per ACT piece
        rawdiag = persist.tile([128, NOWN], f32)
        labels_sb = persist.tile([128, NCHUNK], f32)
        iota_sb = persist.tile([128, NCLS], f32)
        eye33_sb = persist.tile([NCLS, NCLS], f32)
        ones_f = persist.tile([128, 1], f32)
        ones_bf = persist.tile([128, 1], bf16)
        lnhalf_sb = persist.tile([128, 1], f32)
        ones_row = persist.tile([1, 128], f32)
        cnt_row = persist.tile([1, NCLS], f32)
        cnt_bcast = persist.tile([128, NCLS], f32)
        cnt_part = persist.tile([128, NCLS], f32)
        Msum_sb = persist.tile([NCLS, 128], f32)
        Mt_sb = persist.tile([128, NCLS], bf16)
        dump33 = persist.tile([128, NCLS], f32)
        zcol_sb = persist.tile([128, NOWN * 32], f32)
        zrow_sb = persist.tile([128, NOWN], f32)
        tsp_sb = persist.tile([128, NOWN], f32)
        valid_sb = persist.tile([128, NOWN], f32)
        Zd32r = persist.tile([128, NOWN], f32)
        e_diag = persist.tile([128, NOWN], f32)
        S_full = persist.tile([128, NOWN], f32)
        S_excl = persist.tile([128, NOWN], f32)
        P_own = persist.tile([128, NOWN], f32)
        P_pos = persist.tile([128, NOWN], f32)
        P_safe = persist.tile([128, NOWN], f32)
        P_inv = persist.tile([128, NOWN], f32)
        Zsum = persist.tile([128, NOWN], f32)

        # ---------------- prologue ----------------
        nc.gpsimd.dma_start(labels_sb[:], lab_dram[:])
        nc.gpsimd.dma_start(iota_sb[:], iota_dram[:])
        nc.gpsimd.dma_start(eye33_sb[:], eye33_dram[:])
        nc.vector.memset(ones_f[:], 1.0)
        nc.vector.memset(ones_bf[:], 1.0)
        nc.vector.memset(ones_row[:], 1.0)
        nc.vector.memset(lnhalf_sb[:], LNHALF)
        nc.vector.tensor_tensor(
            out=O_bf[:].rearrange("p (c k) -> p c k", k=NCLS),
            in0=iota_sb[:].rearrange("p (a k) -> p a k", a=1)
            .to_broadcast((128, NCHUNK, NCLS)),
            in1=labels_sb[:].to_broadcast((128, NCHUNK, NCLS)),
            op=Alu.is_equal,
        )

        misc_pool = ctx.enter_context(
            tc.tile_pool(name="misc_ps", bufs=1, space="PSUM"))
        misc = misc_pool.tile([128, 512], f32, tag="misc")
        # Msum's 64-matmul accumulation group must own its bank: interleaved
        # start=True matmuls in the same bank wipe has_written state.
        msum_ps = misc_pool.tile([128, 512], f32, tag="msum")
        COLS = 0          # misc[:, 0:256]  column sums (32 per chunk)
        CNT = 384         # misc[0:1, 384:417]
        CNTB = 417        # misc[:, 417:450]
        MT = 451          # misc[:, 451:484]

        with (
            tc.tile_pool(name="main_ps", bufs=2, space="PSUM") as main_ps,
            tc.tile_pool(name="build", bufs=3) as build_pool,
            tc.tile_pool(name="esb", bufs=2) as esb_pool,
        ):
            # ---------------- builds (normalize + transpose + Msum) --------
            xh_halves = {}

            def emit_build(h):
                base = h * HALF
                xs = build_pool.tile([128, HALF], f32, tag="xs")
                nc.sync.dma_start(
                    xs[:].rearrange("p (c d) -> p c d", d=128),
                    x_dram[base:base + HALF, :].rearrange(
                        "(c p) d -> p c d", p=128),
                )
                sq = build_pool.tile([128, HALF], f32, tag="sq")
                nc.gpsimd.tensor_mul(sq[:], xs[:], xs[:])
                ssq = build_pool.tile([128, CH], f32, tag="ssq")
                nc.vector.reduce_sum(
                    ssq[:], sq[:].rearrange("p (c d) -> p c d", d=128), axis=X)
                lns = build_pool.tile([128, CH], f32, tag="lns")
                nc.scalar.activation(lns[:], ssq[:], Act.Ln)
                rinv = build_pool.tile([128, CH], f32, tag="rinv")
                nc.scalar.activation(rinv[:], lns[:], Act.Exp, scale=-0.5)
                xh = build_pool.tile([128, HALF], bf16, tag="xh")
                nc.gpsimd.tensor_tensor(
                    out=xh[:].rearrange("p (c r) -> p c r", r=128),
                    in0=xs[:].rearrange("p (c r) -> p c r", r=128),
                    in1=rinv[:].to_broadcast((128, CH, 128)),
                    op=Alu.mult,
                )
                if h < NXT:
                    nc.sync.dma_start_transpose(
                        xT[:, base:base + HALF].rearrange(
                            "p (c r) -> p c r", r=128),
                        xh[:],
                    )
                if h == 0:
                    sq2 = build_pool.tile([128, HALF], f32, tag="sq2")
                    nc.vector.tensor_mul(sq2[:], xh[:], xh[:])
                    nc.vector.reduce_sum(
                        rawdiag[:],
                        sq2[:].rearrange("p (c d) -> p c d", d=128), axis=X)
                xh_halves[h] = xh

            def emit_msum(h):
                xh = xh_halves.pop(h)
                for i in range(CH):
                    c = h * CH + i
                    nc.tensor.matmul(
                        msum_ps[0:NCLS, 0:128],
                        O_bf[:, c * NCLS:(c + 1) * NCLS],
                        xh[:, i * 128:(i + 1) * 128],
                        start=(c == 0),
                        stop=(c == NCHUNK - 1),
                        skip_group_check=True,
                    )

            for h in range(NXT):
                emit_build(h)
                emit_msum(h)

            # ---------------- main loop: band logits + exp + colsums -------
            prev_esb = None

            def emit_colsum(m, esb):
                for j in range(1, 32):
                    nc.tensor.matmul(
                        misc[:, COLS + 32 * m + (j - 1):COLS + 32 * m + j],
                        esb[:, 128 * j:128 * j + 128],
                        ones_bf[:],
                        start=True, stop=True,
                    )

            for m in range(NOWN):
                lhsT = xT[:, m * 128:(m + 1) * 128]
                esb = esb_pool.tile([128, MAINW], bf16, tag="esb")
                for kp, (off, w) in enumerate(PIECES):
                    ps = main_ps.tile([128, 1536], f32, tag="e")
                    for k in range(w // 512):
                        nc.tensor.matmul(
                            ps[:, k * 512:(k + 1) * 512],
                            lhsT,
                            xT[:, 128 * m + off + k * 512:
                               128 * m + off + (k + 1) * 512],
                            start=True, stop=True,
                        )
                    nc.scalar.activation(
                        esb[:, off:off + w], ps[:, 0:w], Act.Exp,
                        scale=INV_TAU,
                        accum_out=Zacc[:, 3 * m + kp:3 * m + kp + 1],
                    )
                if m > 0:
                    emit_colsum(m - 1, prev_esb)
                if m >= 2 and m <= 4:
                    h = m + 3          # builds 5..7 during main chunks 2..4
                    emit_build(h)
                    emit_msum(h)
                if m == 1:
                    # per-row positive-count pieces; runs in main idle time
                    nc.vector.tensor_tensor(
                        out=O_own[:].rearrange("p (c k) -> p c k", k=NCLS),
                        in0=iota_sb[:].rearrange("p (a k) -> p a k", a=1)
                        .to_broadcast((128, NOWN, NCLS)),
                        in1=labels_sb[:, 0:NOWN].to_broadcast(
                            (128, NOWN, NCLS)),
                        op=Alu.is_equal,
                    )
                    nc.vector.reduce_sum(
                        cnt_part[:],
                        O_bf[:].rearrange("p (c k) -> p k c", k=NCLS), axis=X)
                prev_esb = esb
            emit_colsum(NOWN - 1, prev_esb)

        # ---------------- tail / epilogue ----------------
        with tc.tile_pool(name="epi_ps", bufs=1, space="PSUM") as epi_ps:
            # d32 blocks: logits, exp (x0.5 via bias), row-reduce, colsums
            d32_ps = epi_ps.tile([128, NOWN * 128], f32, tag="d32")
            for m in range(NOWN):
                nc.tensor.matmul(
                    d32_ps[:, 128 * m:128 * m + 128],
                    xT[:, m * 128:(m + 1) * 128],
                    xT[:, 128 * m + MAINW:128 * m + BANDW],
                    start=True, stop=True,
                )
            nc.scalar.activation(
                e_d32[:], d32_ps[:], Act.Exp, scale=INV_TAU, bias=lnhalf_sb[:])
            nc.vector.reduce_sum(
                Zd32r[:], e_d32[:].rearrange("p (m r) -> p m r", r=128), axis=X)
            for m in range(NOWN):
                nc.tensor.matmul(
                    misc[:, COLS + 32 * m + 31:COLS + 32 * m + 32],
                    e_d32[:, 128 * m:128 * m + 128],
                    ones_bf[:],
                    start=True, stop=True,
                )
            nc.vector.tensor_copy(zcol_sb[:], misc[:, COLS:COLS + 256])

            # Z row partials: main accums + d32 - self term
            nc.vector.reduce_sum(
                Zsum[:], Zacc[:].rearrange("p (m k) -> p m k", k=3), axis=X)
            nc.scalar.activation(e_diag[:], rawdiag[:], Act.Exp, scale=INV_TAU)
            nc.vector.tensor_add(zrow_sb[:], Zsum[:], Zd32r[:])
            nc.vector.tensor_sub(zrow_sb[:], zrow_sb[:], e_diag[:])

            # class counts -> P
            cnt_ps = misc[0:1, CNT:CNT + NCLS]
            nc.tensor.matmul(cnt_ps, ones_f[:], cnt_part[:], start=True, stop=True)
            nc.vector.tensor_copy(cnt_row[:], cnt_ps)
            cntb_ps = misc[:, CNTB:CNTB + NCLS]
            nc.tensor.matmul(cntb_ps, ones_row[:], cnt_row[:], start=True, stop=True)
            nc.vector.tensor_copy(cnt_bcast[:], cntb_ps)
            for m in range(NOWN):
                nc.vector.scalar_tensor_tensor(
                    out=dump33[:],
                    in0=O_own[:, m * NCLS:(m + 1) * NCLS],
                    scalar=1.0,
                    in1=cnt_bcast[:],
                    op0=Alu.mult,
                    op1=Alu.mult,
                    accum_out=P_own[:, m:m + 1],
                )

            # S via class sums: F = x_own @ Msum^T, select own class
            nc.vector.tensor_copy(Msum_sb[:], msum_ps[0:NCLS, 0:128])
            mt_ps = misc[:, MT:MT + NCLS]
            nc.tensor.transpose(mt_ps, Msum_sb[:], eye33_sb[:])
            nc.vector.tensor_copy(Mt_sb[:], mt_ps)
            F_ps = epi_ps.tile([128, NOWN * NCLS], f32, tag="F")
            for m in range(NOWN):
                nc.tensor.matmul(
                    F_ps[:, m * NCLS:(m + 1) * NCLS],
                    xT[:, m * 128:(m + 1) * 128],
                    Mt_sb[:],
                    start=True, stop=True,
                )
            for m in range(NOWN):
                nc.vector.scalar_tensor_tensor(
                    out=dump33[:],
                    in0=F_ps[:, m * NCLS:(m + 1) * NCLS],
                    scalar=1.0,
                    in1=O_own[:, m * NCLS:(m + 1) * NCLS],
                    op0=Alu.mult,
                    op1=Alu.mult,
                    accum_out=S_full[:, m:m + 1],
                )

            nc.vector.tensor_sub(S_excl[:], S_full[:], rawdiag[:])
            nc.vector.tensor_scalar_add(P_pos[:], P_own[:], -1.0)
            nc.vector.tensor_scalar_max(P_safe[:], P_pos[:], 1.0)
            nc.vector.reciprocal(P_inv[:], P_safe[:])
            nc.vector.tensor_scalar_min(valid_sb[:], P_pos[:], 1.0)
            nc.vector.scalar_tensor_tensor(
                out=tsp_sb[:], in0=S_excl[:], scalar=INV_TAU, in1=P_inv[:],
                op0=Alu.mult, op1=Alu.mult,
            )

            nc.sync.dma_start(zrow_dram[:], zrow_sb[:])
            nc.sync.dma_start(zcol_dram[:], zcol_sb[:])
            nc.sync.dma_start(tsp_dram[:], tsp_sb[:])
            nc.sync.dma_start(valid_dram[:], valid_sb[:])
            if DEBUG_OUTPUTS:
                nc.sync.dma_start(dbg_msum[:], Msum_sb[:])
                nc.sync.dma_start(dbg_sfull[:], S_full[:])
                nc.sync.dma_start(dbg_pown[:], P_own[:])
                nc.sync.dma_start(dbg_raw[:], rawdiag[:])

    if split_waits:
        _split_multiwait(nc)
    return nc


def _get_nc(split_waits=True):
    global _NC
    if _NC is None:
        _NC = _build_nc(split_waits)
    return _NC


def _make_in_maps(x, lab):
    iota = np.ascontiguousarray(
        np.tile(np.arange(NCLS, dtype=np.float32), (128, 1))
    )
    in_maps = []
    for c in range(N_CORES):
        lo = c * ROWS_PER_CORE
        perm = np.concatenate([np.arange(lo, N), np.arange(0, lo)])
        xp = np.ascontiguousarray(x[perm])
        lp = np.ascontiguousarray(
            lab[perm].astype(np.float32).reshape(NCHUNK, 128).T
        )
        in_maps.append(
            {"xperm": xp, "labels_pc": lp, "iota33": iota,
             "eye33": np.eye(NCLS, dtype=np.float32)}
        )
    return in_maps


def _combine(results):
    Z = np.zeros(N, dtype=np.float64)
    p = np.arange(128)[:, None]
    m = np.arange(NOWN)[None, :]
    idx_row = (128 * m + p).ravel()
    mm = np.arange(NOWN)[None, :, None]
    jj = np.arange(32)[None, None, :]
    pp = np.arange(128)[:, None, None]
    idx_col = (128 * (mm + 1 + jj) + pp).ravel()
    for c in range(N_CORES):
        r = results[c]
        Zloc = np.zeros(N, dtype=np.float64)
        np.add.at(Zloc, idx_row, np.asarray(r["zrow"], dtype=np.float64).ravel())
        np.add.at(Zloc, idx_col,
                  np.asarray(r["zcol"], dtype=np.float64)
                  .reshape(128, NOWN, 32).transpose(0, 1, 2).ravel())
        Z += np.roll(Zloc, ROWS_PER_CORE * c)
    loss_num = 0.0
    nvalid = 0.0
    l_loc = (128 * np.arange(NOWN)[None, :] + np.arange(128)[:, None])
    for c in range(N_CORES):
        r = results[c]
        tsp = np.asarray(r["tsp"], dtype=np.float64)
        val = np.asarray(r["valid"], dtype=np.float64)
        g = (ROWS_PER_CORE * c + l_loc) % N
        lnZ = np.log(Z[g])
        loss_num += ((tsp - lnZ) * val).sum()
        nvalid += val.sum()
    return np.array(-loss_num / nvalid, dtype=np.float32)


def kernel(feature_embeds, label_ids):
    from concourse.bass_utils import run_bass_kernel_spmd

    x = np.asarray(feature_embeds, dtype=np.float32)
    lab = np.asarray(label_ids)
    nc = _get_nc()
    res = run_bass_kernel_spmd(nc, _make_in_maps(x, lab), list(range(N_CORES)))
    return _combine(res.results)


def kernel_profiled(feature_embeds, label_ids):
    """Same as kernel(), but with NTFF tracing; returns (loss, exec_time_ns)."""
    print("ntff hook installed:", _install_ntff_hook())
    from concourse.bass_utils import run_bass_kernel_spmd

    x = np.asarray(feature_embeds, dtype=np.float32)
    lab = np.asarray(label_ids)
    nc = _get_nc()
    res = run_bass_kernel_spmd(
        nc, _make_in_maps(x, lab), list(range(N_CORES)), trace=True
    )
    return _combine(res.results), res.exec_time_ns


# revision 22
# speedup vs baseline: 1.2251x; 1.0769x over previous
"""Supervised contrastive loss (nn_Batch_CL) on 8 Trainium2 NeuronCores.

Math (per the reference):
  x = l2_normalize(feature_embeds)            # [N, D]
  logits = (x @ x.T) / tau                    # tau = 0.1
  Z_i    = sum_{j != i} exp(logits[i, j])
  S_i    = sum_{j != i, l_j == l_i} logits[i, j]
  P_i    = |{j != i : l_j == l_i}|
  per_row_i = S_i / P_i - log Z_i   (if P_i > 0 else 0)
  loss = -sum(per_row) / n_valid

Distribution (symmetric-halving, circulant bands): exp(L) is symmetric, so
each exp needs computing only once.  Global row-chunk i (of 64) computes the
band of column-chunks d = 0..32 (mod 64): 4224 columns.  Row-sums of a band
block cover Z for its rows; column-sums cover Z for its columns (the mirror
block is never computed).  d=32 blocks are computed twice fleet-wide, so
their exp carries bias=ln(1/2).  Core c owns row-chunks 8c..8c+7; its input
is x rotated by 1024c rows, making all band columns local indices
128m..128m+4223 (max 5119) -- the SPMD program is identical on every core.

Per-core kernel:
  - band logits via PE (bf16) in [128,1024] PSUM pieces (2-slot ping-pong),
    exp+row-sum fused in ACT via accum_out, exp values -> SBUF bf16.
  - column sums on a global 512-column grid: psum piece rows [1,512] packed
    4-per-bank at partition offsets {0,32,64,96}; banks zeroed once by a
    zeros-matmul, then every colsum matmul (ones[128,1] stationary, wide
    e-slice moving) accumulates with start=False.  Nothing else ever writes
    those banks (a foreign start=True matmul in the same bank wipes
    has_written state and corrupts open accumulations).
  - positive-pair sums via class aggregation (Msum = x_hat^T @ onehot) as a
    single PSUM accumulation over all 64 chunks in its own bank.
  - l2 normalization: squaring on GPSIMD, reduce+scale on DVE, rsqrt =
    Exp(-.5 Ln) on ACT (stays in the natural_log_exp table set).
Host epilogue assembles Z from the row/col partials (rolled by each core's
rotation), then loss = -sum(valid*(S/P/tau - ln Z)) / n_valid.
"""

import numpy as np

N = 8192
D = 128
N_CORES = 8
ROWS_PER_CORE = N // N_CORES          # 1024
NCHUNK = N // 128                     # 64 chunks of 128 rows
NOWN = 8                              # own row-chunks per core
NHALF = 8                             # 1024-row build halves
HALF = 1024
CH = HALF // 128                      # chunks per half (8)
NXT = 5                               # halves that need transposing (band cols)
XTW = 5120                            # xT width (max band col + 1)
BANDW = 4224                          # band width per chunk (d=0..32)
MAINW = 4096                          # band minus the d32 block
NPIECE = 4                            # ACT pieces per chunk, 1024 wide
NCLS = 33
INV_TAU = 10.0
LNHALF = float(np.log(0.5))
DEBUG_OUTPUTS = False

# colsum matmul table: per chunk, (piece t, out col, e_sb offset, width)
def _colsum_table(m):
    g0, g1 = 128 * m + 128, 128 * m + MAINW
    out = []
    for t in range(g0 // 512, (g1 - 1) // 512 + 1):
        lo, hi = max(512 * t, g0), min(512 * (t + 1), g1)
        out.append((t, lo - 512 * t, lo - 128 * m, hi - lo))
    return out

_NC = None

# ---------------------------------------------------------------------------
# Inlined workarounds (kernel.py must be self-contained).
#
# The local walrus build accepts at most ONE sync-wait command per
# instruction (any type). Tile's scheduler attaches several. Two fixes:
#   1. TileContext._drain_and_barrier is replaced so the exit drain's many
#      waits are split across single-wait nops.
#   2. split_multiwait(nc): post-pass that hoists extra sync waits from any
#      instruction onto injected same-engine EventSemaphore instructions
#      placed immediately before it (engines are in-order, so this is
#      semantically identical).
# ---------------------------------------------------------------------------

_nop_counter = [0]


def _split_drain_and_barrier(self, tick_clock, wait_clock):
    import bass_rust

    vec = tick_clock.global_clock  # VectorClock
    for proc in range(len(vec)):
        tickv = vec[proc]
        if tickv > 0:
            nop_inst = self.nc.sync.nop(nofuse=True)
            c = bass_rust.ScopedClock()
            c.require_at_least(None, proc, tickv)
            wait_clock.add_sem_waits(nop_inst.ins, c)
    self.nc.sync.drain()
    self.nc.all_engine_barrier()
    assert self.sems is not None
    popped = self.nc._tile_sem_poison_stack.pop()
    assert popped is self._sem_poison
    self.nc.clear_and_free_semaphores(list(self.sems.allocated().values()))
    self.nc.all_engine_barrier()


def _install_tile_patch():
    from concourse import tile as _tile

    _tile.TileContext._drain_and_barrier = _split_drain_and_barrier


def _split_multiwait(nc):
    """Hoist all-but-one sync wait from every instruction onto nops."""
    import concourse.mybir as mybir

    n_hoisted = 0
    for bb in nc.main_func.blocks:
        insns = bb.instructions
        out = []
        changed = False
        for ins in insns:
            si = ins.sync_info
            if si is not None and len(si.on_wait) > 1:
                waits = list(si.on_wait)
                for w in waits[:-1]:
                    _nop_counter[0] += 1
                    nop = mybir.InstEventSemaphore(
                        name=f"hoistnop-{_nop_counter[0]}",
                        engine=ins.engine,
                        sync_info=mybir.SyncInfo(on_wait=[w], on_update=[]),
                    )
                    out.append(nop)
                    n_hoisted += 1
                ins.sync_info = mybir.SyncInfo(
                    on_wait=[waits[-1]], on_update=list(si.on_update)
                )
                changed = True
            out.append(ins)
        if changed:
            bb.instructions = out
    return n_hoisted


def _install_ntff_hook():
    """Synthesize the antenv.axon_hooks module missing from this image so
    run_bass_kernel_spmd(trace=True) can NTFF-profile under axon."""
    import sys
    import types

    if "antenv.axon_hooks" in sys.modules:
        return True
    try:
        import antenv
        from trn_agent_boot.trn_boot import _ntff_profile_via_ctypes
    except ImportError:
        return False
    hook_box = [None]
    mod = types.ModuleType("antenv.axon_hooks")
    mod.set_axon_ntff_profile_hook = lambda h: hook_box.__setitem__(0, h)
    mod.get_axon_ntff_profile_hook = lambda: hook_box[0]
    sys.modules["antenv.axon_hooks"] = mod
    antenv.axon_hooks = mod
    hook = _ntff_profile_via_ctypes("/opt/axon/libaxon_pjrt.so")
    mod.set_axon_ntff_profile_hook(hook)
    return hook is not None


def _build_nc(split_waits=True):
    import concourse.bass as bass
    import concourse.mybir as mybir
    from concourse import tile
    from contextlib import ExitStack

    _install_tile_patch()

    f32 = mybir.dt.float32
    bf16 = mybir.dt.bfloat16
    Alu = mybir.AluOpType
    Act = mybir.ActivationFunctionType
    X = mybir.AxisListType.X

    nc = bass.Bass()
    x_dram = nc.dram_tensor("xperm", [N, D], f32, kind="ExternalInput")
    lab_dram = nc.dram_tensor("labels_pc", [128, NCHUNK], f32, kind="ExternalInput")
    iota_dram = nc.dram_tensor("iota33", [128, NCLS], f32, kind="ExternalInput")
    eye33_dram = nc.dram_tensor("eye33", [NCLS, NCLS], f32, kind="ExternalInput")
    zrow_dram = nc.dram_tensor("zrow", [128, NOWN], f32, kind="ExternalOutput")
    zcol_dram = nc.dram_tensor("zcol", [128, 4 * 512], f32, kind="ExternalOutput")
    tsp_dram = nc.dram_tensor("tsp", [128, NOWN], f32, kind="ExternalOutput")
    valid_dram = nc.dram_tensor("valid", [128, NOWN], f32, kind="ExternalOutput")
    if DEBUG_OUTPUTS:
        dbg_msum = nc.dram_tensor("dbg_msum", [NCLS, 128], f32, kind="ExternalOutput")
        dbg_sfull = nc.dram_tensor("dbg_sfull", [128, NOWN], f32, kind="ExternalOutput")
        dbg_pown = nc.dram_tensor("dbg_pown", [128, NOWN], f32, kind="ExternalOutput")
        dbg_raw = nc.dram_tensor("dbg_raw", [128, NOWN], f32, kind="ExternalOutput")

    with tile.TileContext(nc) as tc, ExitStack() as ctx:
        persist = ctx.enter_context(tc.tile_pool(name="persist", bufs=1))

        xT = persist.tile([128, XTW], bf16)               # normalized, transposed
        xh57 = persist.tile([128, 3 * HALF], bf16)        # halves 5-7 (Msum only)
        e_d32 = persist.tile([128, NOWN * 128], bf16)     # exp of d32 blocks (x0.5)
        O_bf = persist.tile([128, NCHUNK * NCLS], bf16)   # one-hot labels (PE operand)
        O_own = persist.tile([128, NOWN * NCLS], f32)     # one-hot, own chunks (DVE)
        Zacc = persist.tile([128, NPIECE * NOWN], f32)    # exp accum per ACT piece
        rawdiag = persist.tile([128, NOWN], f32)
        labels_sb = persist.tile([128, NCHUNK], f32)
        iota_sb = persist.tile([128, NCLS], f32)
        eye33_sb = persist.tile([NCLS, NCLS], f32)
        ones_f = persist.tile([128, 1], f32)
        ones_bf = persist.tile([128, 1], bf16)
        lnhalf_sb = persist.tile([128, 1], f32)
        zeros512 = persist.tile([128, 512], bf16)
        ones_row = persist.tile([1, 128], f32)
        cnt_row = persist.tile([1, NCLS], f32)
        cnt_bcast = persist.tile([128, NCLS], f32)
        cnt_part = persist.tile([128, NCLS], f32)
        Msum_sb = persist.tile([NCLS, 128], f32)
        Mt_sb = persist.tile([128, NCLS], bf16)
        dump33 = persist.tile([128, NCLS], f32)
        zcol_sb = persist.tile([128, 4 * 512], f32)
        zrow_sb = persist.tile([128, NOWN], f32)
        tsp_sb = persist.tile([128, NOWN], f32)
        valid_sb = persist.tile([128, NOWN], f32)
        Zd32r = persist.tile([128, NOWN], f32)
        e_diag = persist.tile([128, NOWN], f32)
        S_full = persist.tile([128, NOWN], f32)
        S_excl = persist.tile([128, NOWN], f32)
        P_own = persist.tile([128, NOWN], f32)
        P_pos = persist.tile([128, NOWN], f32)
        P_safe = persist.tile([128, NOWN], f32)
        P_inv = persist.tile([128, NOWN], f32)
        Zsum = persist.tile([128, NOWN], f32)

        # persistent PSUM: 3 colsum banks + 1 Msum bank
        glob_ps = ctx.enter_context(
            tc.tile_pool(name="glob_ps", bufs=1, space="PSUM"))
        cs_banks = [glob_ps.tile([128, 512], f32, tag=f"cs{b}", name=f"cs{b}")
                    for b in range(3)]
        msum_ps = glob_ps.tile([128, 512], f32, tag="msum")

        def cs_slot(t):
            # base_partition() allows only {0,32,64}: 3 piece-rows per bank.
            # Piece 9 borrows the Msum bank's row 64 (Msum sits at rows 0-32;
            # its single start=True precedes every piece-9 write).
            if t == 9:
                return msum_ps, 64
            return cs_banks[t // 3], 32 * (t % 3)

        # ---------------- prologue smalls ----------------
        nc.gpsimd.dma_start(labels_sb[:], lab_dram[:])
        nc.gpsimd.dma_start(iota_sb[:], iota_dram[:])
        nc.gpsimd.dma_start(eye33_sb[:], eye33_dram[:])
        nc.vector.memset(ones_f[:], 1.0)
        nc.vector.memset(ones_bf[:], 1.0)
        nc.vector.memset(ones_row[:], 1.0)
        nc.vector.memset(lnhalf_sb[:], LNHALF)
        nc.vector.memset(zeros512[:], 0.0)
        # zero the colsum accumulator banks (sets has_written everywhere)
        for b in range(3):
            nc.tensor.matmul(cs_banks[b][:, 0:512], zeros512[:, 0:128],
                             zeros512[:], start=True, stop=True)
        nc.tensor.matmul(msum_ps[64:65, 0:512], ones_bf[:],
                         zeros512[:], start=True, stop=True)
        nc.vector.tensor_tensor(
            out=O_bf[:].rearrange("p (c k) -> p c k", k=NCLS),
            in0=iota_sb[:].rearrange("p (a k) -> p a k", a=1)
            .to_broadcast((128, NCHUNK, NCLS)),
            in1=labels_sb[:].to_broadcast((128, NCHUNK, NCLS)),
            op=Alu.is_equal,
        )

        with (
            tc.tile_pool(name="main_ps", bufs=2, space="PSUM") as main_ps,
            tc.tile_pool(name="build", bufs=3) as build_pool,
            tc.tile_pool(name="esb", bufs=2) as esb_pool,
        ):
            # ---------------- builds (normalize + transpose + Msum) --------
            xh_halves = {}

            def emit_build(h):
                base = h * HALF
                xs = build_pool.tile([128, HALF], f32, tag="xs")
                nc.sync.dma_start(
                    xs[:].rearrange("p (c d) -> p c d", d=128),
                    x_dram[base:base + HALF, :].rearrange(
                        "(c p) d -> p c d", p=128),
                )
                sq = build_pool.tile([128, HALF], f32, tag="sq")
                nc.gpsimd.tensor_mul(sq[:], xs[:], xs[:])
                ssq = build_pool.tile([128, CH], f32, tag="ssq")
                nc.vector.reduce_sum(
                    ssq[:], sq[:].rearrange("p (c d) -> p c d", d=128), axis=X)
                lns = build_pool.tile([128, CH], f32, tag="lns")
                nc.scalar.activation(lns[:], ssq[:], Act.Ln)
                rinv = build_pool.tile([128, CH], f32, tag="rinv")
                nc.scalar.activation(rinv[:], lns[:], Act.Exp, scale=-0.5)
                if h < NXT:
                    xh = build_pool.tile([128, HALF], bf16, tag="xh")
                else:
                    xh = xh57[:, (h - NXT) * HALF:(h - NXT + 1) * HALF]
                nc.vector.scalar_tensor_tensor(
                    out=xh.rearrange("p (c r) -> p c r", r=128),
                    in0=xs[:].rearrange("p (c r) -> p c r", r=128),
                    scalar=1.0,
                    in1=rinv[:].to_broadcast((128, CH, 128)),
                    op0=Alu.mult,
                    op1=Alu.mult,
                )
                if h < NXT:
                    nc.scalar.dma_start_transpose(
                        xT[:, base:base + HALF].rearrange(
                            "p (c r) -> p c r", r=128),
                        xh,
                    )
                if h == 0:
                    sq2 = build_pool.tile([128, HALF], f32, tag="sq2")
                    nc.vector.tensor_mul(sq2[:], xh, xh)
                    nc.vector.reduce_sum(
                        rawdiag[:],
                        sq2[:].rearrange("p (c d) -> p c d", d=128), axis=X)
                xh_halves[h] = xh

            def emit_msum(h):
                xh = xh_halves.pop(h)
                for i in range(CH):
                    c = h * CH + i
                    nc.tensor.matmul(
                        msum_ps[0:NCLS, 0:128],
                        O_bf[:, c * NCLS:(c + 1) * NCLS],
                        xh[:, i * 128:(i + 1) * 128],
                        start=(c == 0),
                        stop=(c == NCHUNK - 1),
                        skip_group_check=True,
                    )

            for h in range(NXT):
                emit_build(h)
                emit_msum(h)

            # ---------------- main loop: band logits + exp + colsums -------
            prev_esb = None

            def emit_colsum(m, esb):
                for t, outc, eoff, w in _colsum_table(m):
                    bank, row = cs_slot(t)
                    nc.tensor.matmul(
                        bank[row:row + 1, outc:outc + w],
                        ones_bf[:],
                        esb[:, eoff:eoff + w],
                        start=False, stop=True,
                        skip_group_check=True,
                    )

            for m in range(NOWN):
                lhsT = xT[:, m * 128:(m + 1) * 128]
                esb = esb_pool.tile([128, MAINW], bf16, tag="esb")
                for kp in range(NPIECE):
                    off = kp * 1024
                    ps = main_ps.tile([128, 1024], f32, tag="e")
                    for k in range(2):
                        nc.tensor.matmul(
                            ps[:, k * 512:(k + 1) * 512],
                            lhsT,
                            xT[:, 128 * m + off + k * 512:
                               128 * m + off + (k + 1) * 512],
                            start=True, stop=True,
                        )
                    nc.scalar.activation(
                        esb[:, off:off + 1024], ps[:], Act.Exp,
                        scale=INV_TAU,
                        accum_out=Zacc[:, NPIECE * m + kp:NPIECE * m + kp + 1],
                    )
                if m > 0:
                    emit_colsum(m - 1, prev_esb)
                if m >= 2 and m <= 4:
                    h = m + 3          # builds 5..7 during main chunks 2..4
                    emit_build(h)
                    emit_msum(h)
                if m == 1:
                    # per-row positive-count pieces; runs in main idle time
                    nc.vector.tensor_tensor(
                        out=O_own[:].rearrange("p (c k) -> p c k", k=NCLS),
                        in0=iota_sb[:].rearrange("p (a k) -> p a k", a=1)
                        .to_broadcast((128, NOWN, NCLS)),
                        in1=labels_sb[:, 0:NOWN].to_broadcast(
                            (128, NOWN, NCLS)),
                        op=Alu.is_equal,
                    )
                    nc.vector.reduce_sum(
                        cnt_part[:],
                        O_bf[:].rearrange("p (c k) -> p k c", k=NCLS), axis=X)
                prev_esb = esb
            emit_colsum(NOWN - 1, prev_esb)

        # ---------------- tail / epilogue ----------------
        with tc.tile_pool(name="epi_ps", bufs=1, space="PSUM") as epi_ps:
            # d32 blocks: logits, exp (x0.5 via bias), row-reduce, colsums
            d32_ps = epi_ps.tile([128, NOWN * 128], f32, tag="d32")
            for m in range(NOWN):
                nc.tensor.matmul(
                    d32_ps[:, 128 * m:128 * m + 128],
                    xT[:, m * 128:(m + 1) * 128],
                    xT[:, 128 * m + MAINW:128 * m + BANDW],
                    start=True, stop=True,
                )
            nc.scalar.activation(
                e_d32[:], d32_ps[:], Act.Exp, scale=INV_TAU, bias=lnhalf_sb[:])
            nc.vector.reduce_sum(
                Zd32r[:], e_d32[:].rearrange("p (m r) -> p m r", r=128), axis=X)
            for m in range(NOWN):
                g = 128 * m + MAINW
                bank, row = cs_slot(g // 512)
                outc = g - 512 * (g // 512)
                nc.tensor.matmul(
                    bank[row:row + 1, outc:outc + 128],
                    ones_bf[:],
                    e_d32[:, 128 * m:128 * m + 128],
                    start=False, stop=True,
                    skip_group_check=True,
                )
            for b in range(3):
                nc.vector.tensor_copy(
                    zcol_sb[:, 512 * b:512 * (b + 1)], cs_banks[b][:])
            nc.vector.tensor_copy(zcol_sb[:, 1536:2048], msum_ps[:, 0:512])

            # Z row partials: main accums + d32 - self term
            nc.vector.reduce_sum(
                Zsum[:], Zacc[:].rearrange("p (m k) -> p m k", k=NPIECE), axis=X)
            nc.scalar.activation(e_diag[:], rawdiag[:], Act.Exp, scale=INV_TAU)
            nc.vector.tensor_add(zrow_sb[:], Zsum[:], Zd32r[:])
            nc.vector.tensor_sub(zrow_sb[:], zrow_sb[:], e_diag[:])

            # class counts -> P
            smalls = epi_ps.tile([128, 512], f32, tag="smalls")
            cnt_ps = smalls[0:1, 0:NCLS]
            nc.tensor.matmul(cnt_ps, ones_f[:], cnt_part[:], start=True, stop=True)
            nc.vector.tensor_copy(cnt_row[:], cnt_ps)
            cntb_ps = smalls[:, 64:64 + NCLS]
            nc.tensor.matmul(cntb_ps, ones_row[:], cnt_row[:], start=True, stop=True)
            nc.vector.tensor_copy(cnt_bcast[:], cntb_ps)
            for m in range(NOWN):
                nc.vector.scalar_tensor_tensor(
                    out=dump33[:],
                    in0=O_own[:, m * NCLS:(m + 1) * NCLS],
                    scalar=1.0,
                    in1=cnt_bcast[:],
                    op0=Alu.mult,
                    op1=Alu.mult,
                    accum_out=P_own[:, m:m + 1],
                )

            # S via class sums: F = x_own @ Msum^T, select own class
            nc.vector.tensor_copy(Msum_sb[:], msum_ps[0:NCLS, 0:128])
            mt_ps = smalls[:, 128:128 + NCLS]
            nc.tensor.transpose(mt_ps, Msum_sb[:], eye33_sb[:])
            nc.vector.tensor_copy(Mt_sb[:], mt_ps)
            F_ps = epi_ps.tile([128, NOWN * NCLS], f32, tag="F")
            for m in range(NOWN):
                nc.tensor.matmul(
                    F_ps[:, m * NCLS:(m + 1) * NCLS],
                    xT[:, m * 128:(m + 1) * 128],
                    Mt_sb[:],
                    start=True, stop=True,
                )
            for m in range(NOWN):
                nc.vector.scalar_tensor_tensor(
                    out=dump33[:],
                    in0=F_ps[:, m * NCLS:(m + 1) * NCLS],
                    scalar=1.0,
                    in1=O_own[:, m * NCLS:(m + 1) * NCLS],
                    op0=Alu.mult,
                    op1=Alu.mult,
                    accum_out=S_full[:, m:m + 1],
                )

            nc.vector.tensor_sub(S_excl[:], S_full[:], rawdiag[:])
            nc.vector.tensor_scalar_add(P_pos[:], P_own[:], -1.0)
            nc.vector.tensor_scalar_max(P_safe[:], P_pos[:], 1.0)
            nc.vector.reciprocal(P_inv[:], P_safe[:])
            nc.vector.tensor_scalar_min(valid_sb[:], P_pos[:], 1.0)
            nc.vector.scalar_tensor_tensor(
                out=tsp_sb[:], in0=S_excl[:], scalar=INV_TAU, in1=P_inv[:],
                op0=Alu.mult, op1=Alu.mult,
            )

            nc.sync.dma_start(zrow_dram[:], zrow_sb[:])
            nc.sync.dma_start(zcol_dram[:], zcol_sb[:])
            nc.sync.dma_start(tsp_dram[:], tsp_sb[:])
            nc.sync.dma_start(valid_dram[:], valid_sb[:])
            if DEBUG_OUTPUTS:
                nc.sync.dma_start(dbg_msum[:], Msum_sb[:])
                nc.sync.dma_start(dbg_sfull[:], S_full[:])
                nc.sync.dma_start(dbg_pown[:], P_own[:])
                nc.sync.dma_start(dbg_raw[:], rawdiag[:])

    if split_waits:
        _split_multiwait(nc)
    return nc


def _get_nc(split_waits=True):
    global _NC
    if _NC is None:
        _NC = _build_nc(split_waits)
    return _NC


def _make_in_maps(x, lab):
    iota = np.ascontiguousarray(
        np.tile(np.arange(NCLS, dtype=np.float32), (128, 1))
    )
    in_maps = []
    for c in range(N_CORES):
        lo = c * ROWS_PER_CORE
        perm = np.concatenate([np.arange(lo, N), np.arange(0, lo)])
        xp = np.ascontiguousarray(x[perm])
        lp = np.ascontiguousarray(
            lab[perm].astype(np.float32).reshape(NCHUNK, 128).T
        )
        in_maps.append(
            {"xperm": xp, "labels_pc": lp, "iota33": iota,
             "eye33": np.eye(NCLS, dtype=np.float32)}
        )
    return in_maps


def _combine(results):
    Z = np.zeros(N, dtype=np.float64)
    p = np.arange(128)[:, None]
    m = np.arange(NOWN)[None, :]
    idx_row = (128 * m + p).ravel()
    for c in range(N_CORES):
        r = results[c]
        Zloc = np.zeros(N, dtype=np.float64)
        np.add.at(Zloc, idx_row, np.asarray(r["zrow"], dtype=np.float64).ravel())
        zc = np.asarray(r["zcol"], dtype=np.float64)
        # pieces 0-8: [32*(t%3), 512*(t//3)+c]; piece 9: [64, 1536+c]
        for t in range(10):
            if t == 9:
                colsum = zc[64, 1536:2048]
            else:
                colsum = zc[32 * (t % 3), 512 * (t // 3):512 * (t // 3) + 512]
            lo = max(128, 512 * t)
            hi = min(XTW, 512 * (t + 1))
            Zloc[lo:hi] += colsum[lo - 512 * t:hi - 512 * t]
        Z += np.roll(Zloc, ROWS_PER_CORE * c)
    loss_num = 0.0
    nvalid = 0.0
    l_loc = (128 * np.arange(NOWN)[None, :] + np.arange(128)[:, None])
    for c in range(N_CORES):
        r = results[c]
        tsp = np.asarray(r["tsp"], dtype=np.float64)
        val = np.asarray(r["valid"], dtype=np.float64)
        g = (ROWS_PER_CORE * c + l_loc) % N
        lnZ = np.log(Z[g])
        loss_num += ((tsp - lnZ) * val).sum()
        nvalid += val.sum()
    return np.array(-loss_num / nvalid, dtype=np.float32)


def kernel(feature_embeds, label_ids):
    from concourse.bass_utils import run_bass_kernel_spmd

    x = np.asarray(feature_embeds, dtype=np.float32)
    lab = np.asarray(label_ids)
    nc = _get_nc()
    res = run_bass_kernel_spmd(nc, _make_in_maps(x, lab), list(range(N_CORES)))
    return _combine(res.results)


def kernel_profiled(feature_embeds, label_ids):
    """Same as kernel(), but with NTFF tracing; returns (loss, exec_time_ns)."""
    print("ntff hook installed:", _install_ntff_hook())
    from concourse.bass_utils import run_bass_kernel_spmd

    x = np.asarray(feature_embeds, dtype=np.float32)
    lab = np.asarray(label_ids)
    nc = _get_nc()
    res = run_bass_kernel_spmd(
        nc, _make_in_maps(x, lab), list(range(N_CORES)), trace=True
    )
    return _combine(res.results), res.exec_time_ns


# revision 28
# speedup vs baseline: 1.2866x; 1.0502x over previous
"""Supervised contrastive loss (nn_Batch_CL) on 8 Trainium2 NeuronCores.

Math (per the reference):
  x = l2_normalize(feature_embeds)            # [N, D]
  logits = (x @ x.T) / tau                    # tau = 0.1
  Z_i    = sum_{j != i} exp(logits[i, j])
  S_i    = sum_{j != i, l_j == l_i} logits[i, j]
  P_i    = |{j != i : l_j == l_i}|
  per_row_i = S_i / P_i - log Z_i   (if P_i > 0 else 0)
  loss = -sum(per_row) / n_valid

Distribution (symmetric-halving, circulant bands): exp(L) is symmetric, so
each exp needs computing only once.  Global row-chunk i (of 64) computes the
band of column-chunks d = 0..32 (mod 64): 4224 columns.  Row-sums of a band
block cover Z for its rows; column-sums cover Z for its columns (the mirror
block is never computed).  d=32 blocks are computed twice fleet-wide, so
their exp carries bias=ln(1/2).  Core c owns row-chunks 8c..8c+7; its input
is x rotated by 1024c rows, making all band columns local indices
128m..128m+4223 (max 5119) -- the SPMD program is identical on every core.

Per-core kernel:
  - band logits via PE (bf16) in [128,1024] PSUM pieces (2-slot ping-pong),
    exp+row-sum fused in ACT via accum_out, exp values -> SBUF bf16.
  - column sums on a global 512-column grid: psum piece rows [1,512] packed
    4-per-bank at partition offsets {0,32,64,96}; banks zeroed once by a
    zeros-matmul, then every colsum matmul (ones[128,1] stationary, wide
    e-slice moving) accumulates with start=False.  Nothing else ever writes
    those banks (a foreign start=True matmul in the same bank wipes
    has_written state and corrupts open accumulations).
  - positive-pair sums via class aggregation (Msum = x_hat^T @ onehot) as a
    single PSUM accumulation over all 64 chunks in its own bank.
  - l2 normalization: squaring on GPSIMD, reduce+scale on DVE, rsqrt =
    Exp(-.5 Ln) on ACT (stays in the natural_log_exp table set).
Host epilogue assembles Z from the row/col partials (rolled by each core's
rotation), then loss = -sum(valid*(S/P/tau - ln Z)) / n_valid.
"""

import numpy as np

N = 8192
D = 128
N_CORES = 8
ROWS_PER_CORE = N // N_CORES          # 1024
NCHUNK = N // 128                     # 64 chunks of 128 rows
NOWN = 8                              # own row-chunks per core
NHALF = 8                             # 1024-row build halves
HALF = 1024
CH = HALF // 128                      # chunks per half (8)
NXT = 5                               # halves that need transposing (band cols)
XTW = 5120                            # xT width (max band col + 1)
BANDW = 4224                          # band width per chunk (d=0..32)
MAINW = 4096                          # band minus the d32 block
NPIECE = 4                            # ACT pieces per chunk, 1024 wide
NCLS = 33
INV_TAU = 10.0
LNHALF = float(np.log(0.5))
DEBUG_OUTPUTS = False

# colsum matmul table: per chunk, (piece t, out col, e_sb offset, width)
def _colsum_table(m):
    g0, g1 = 128 * m + 128, 128 * m + MAINW
    out = []
    for t in range(g0 // 512, (g1 - 1) // 512 + 1):
        lo, hi = max(512 * t, g0), min(512 * (t + 1), g1)
        out.append((t, lo - 512 * t, lo - 128 * m, hi - lo))
    return out

_NC = None

# ---------------------------------------------------------------------------
# Inlined workarounds (kernel.py must be self-contained).
#
# The local walrus build accepts at most ONE sync-wait command per
# instruction (any type). Tile's scheduler attaches several. Two fixes:
#   1. TileContext._drain_and_barrier is replaced so the exit drain's many
#      waits are split across single-wait nops.
#   2. split_multiwait(nc): post-pass that hoists extra sync waits from any
#      instruction onto injected same-engine EventSemaphore instructions
#      placed immediately before it (engines are in-order, so this is
#      semantically identical).
# ---------------------------------------------------------------------------

_nop_counter = [0]


def _split_drain_and_barrier(self, tick_clock, wait_clock):
    import bass_rust

    vec = tick_clock.global_clock  # VectorClock
    for proc in range(len(vec)):
        tickv = vec[proc]
        if tickv > 0:
            nop_inst = self.nc.sync.nop(nofuse=True)
            c = bass_rust.ScopedClock()
            c.require_at_least(None, proc, tickv)
            wait_clock.add_sem_waits(nop_inst.ins, c)
    self.nc.sync.drain()
    self.nc.all_engine_barrier()
    assert self.sems is not None
    popped = self.nc._tile_sem_poison_stack.pop()
    assert popped is self._sem_poison
    self.nc.clear_and_free_semaphores(list(self.sems.allocated().values()))
    self.nc.all_engine_barrier()


def _install_tile_patch():
    from concourse import tile as _tile

    _tile.TileContext._drain_and_barrier = _split_drain_and_barrier


def _split_multiwait(nc):
    """Hoist all-but-one sync wait from every instruction onto nops."""
    import concourse.mybir as mybir

    n_hoisted = 0
    for bb in nc.main_func.blocks:
        insns = bb.instructions
        out = []
        changed = False
        for ins in insns:
            si = ins.sync_info
            if si is not None and len(si.on_wait) > 1:
                waits = list(si.on_wait)
                for w in waits[:-1]:
                    _nop_counter[0] += 1
                    nop = mybir.InstEventSemaphore(
                        name=f"hoistnop-{_nop_counter[0]}",
                        engine=ins.engine,
                        sync_info=mybir.SyncInfo(on_wait=[w], on_update=[]),
                    )
                    out.append(nop)
                    n_hoisted += 1
                ins.sync_info = mybir.SyncInfo(
                    on_wait=[waits[-1]], on_update=list(si.on_update)
                )
                changed = True
            out.append(ins)
        if changed:
            bb.instructions = out
    return n_hoisted


def _install_ntff_hook():
    """Synthesize the antenv.axon_hooks module missing from this image so
    run_bass_kernel_spmd(trace=True) can NTFF-profile under axon."""
    import sys
    import types

    if "antenv.axon_hooks" in sys.modules:
        return True
    try:
        import antenv
        from trn_agent_boot.trn_boot import _ntff_profile_via_ctypes
    except ImportError:
        return False
    hook_box = [None]
    mod = types.ModuleType("antenv.axon_hooks")
    mod.set_axon_ntff_profile_hook = lambda h: hook_box.__setitem__(0, h)
    mod.get_axon_ntff_profile_hook = lambda: hook_box[0]
    sys.modules["antenv.axon_hooks"] = mod
    antenv.axon_hooks = mod
    hook = _ntff_profile_via_ctypes("/opt/axon/libaxon_pjrt.so")
    mod.set_axon_ntff_profile_hook(hook)
    return hook is not None


def _build_nc(split_waits=True):
    import concourse.bass as bass
    import concourse.mybir as mybir
    from concourse import tile
    from contextlib import ExitStack

    _install_tile_patch()

    f32 = mybir.dt.float32
    bf16 = mybir.dt.bfloat16
    Alu = mybir.AluOpType
    Act = mybir.ActivationFunctionType
    X = mybir.AxisListType.X

    nc = bass.Bass()
    x_dram = nc.dram_tensor("xperm", [N, D], f32, kind="ExternalInput")
    lab_dram = nc.dram_tensor("labels_pc", [128, NCHUNK], f32, kind="ExternalInput")
    iota_dram = nc.dram_tensor("iota33", [128, NCLS], f32, kind="ExternalInput")
    eye33_dram = nc.dram_tensor("eye33", [NCLS, NCLS], f32, kind="ExternalInput")
    zrow_dram = nc.dram_tensor("zrow", [128, NOWN], f32, kind="ExternalOutput")
    zcol_dram = nc.dram_tensor("zcol", [128, 4 * 512], f32, kind="ExternalOutput")
    f_dram = nc.dram_tensor("fcls", [128, NOWN * NCLS], f32, kind="ExternalOutput")
    raw_dram = nc.dram_tensor("raw", [128, NOWN], f32, kind="ExternalOutput")

    with tile.TileContext(nc) as tc, ExitStack() as ctx:
        persist = ctx.enter_context(tc.tile_pool(name="persist", bufs=1))

        xT = persist.tile([128, XTW], bf16)               # normalized, transposed
        xh57 = persist.tile([128, 3 * HALF], bf16)        # halves 5-7 (Msum only)
        e_d32 = persist.tile([128, NOWN * 128], bf16)     # exp of d32 blocks (x0.5)
        O_bf = persist.tile([128, NCHUNK * NCLS], bf16)   # one-hot labels (PE operand)
        Zacc = persist.tile([128, NPIECE * NOWN], f32)    # exp accum per ACT piece
        rawdiag = persist.tile([128, NOWN], f32)
        labels_sb = persist.tile([128, NCHUNK], f32)
        iota_sb = persist.tile([128, NCLS], f32)
        eye33_sb = persist.tile([NCLS, NCLS], f32)
        ones_bf = persist.tile([128, 1], bf16)
        lnhalf_sb = persist.tile([128, 1], f32)
        zeros512 = persist.tile([128, 512], bf16)
        Msum_sb = persist.tile([NCLS, 128], f32)
        Mt_sb = persist.tile([128, NCLS], bf16)
        zcol_sb = persist.tile([128, 4 * 512], f32)
        zrow_sb = persist.tile([128, NOWN], f32)
        F_sb = persist.tile([128, NOWN * NCLS], f32)
        Zd32r = persist.tile([128, NOWN], f32)
        Zsum = persist.tile([128, NOWN], f32)

        # persistent PSUM: 3 colsum banks + 1 Msum bank
        glob_ps = ctx.enter_context(
            tc.tile_pool(name="glob_ps", bufs=1, space="PSUM"))
        cs_banks = [glob_ps.tile([128, 512], f32, tag=f"cs{b}", name=f"cs{b}")
                    for b in range(3)]
        msum_ps = glob_ps.tile([128, 512], f32, tag="msum")

        def cs_slot(t):
            # base_partition() allows only {0,32,64}: 3 piece-rows per bank.
            # Piece 9 borrows the Msum bank's row 64 (Msum sits at rows 0-32;
            # its single start=True precedes every piece-9 write).
            if t == 9:
                return msum_ps, 64
            return cs_banks[t // 3], 32 * (t % 3)

        # ---------------- prologue smalls ----------------
        nc.gpsimd.dma_start(labels_sb[:], lab_dram[:])
        nc.gpsimd.dma_start(iota_sb[:], iota_dram[:])
        nc.gpsimd.dma_start(eye33_sb[:], eye33_dram[:])
        nc.vector.memset(ones_bf[:], 1.0)
        nc.vector.memset(lnhalf_sb[:], LNHALF)
        nc.vector.memset(zeros512[:], 0.0)
        # zero the colsum accumulator banks (sets has_written everywhere)
        for b in range(3):
            nc.tensor.matmul(cs_banks[b][:, 0:512], zeros512[:, 0:128],
                             zeros512[:], start=True, stop=True)
        nc.tensor.matmul(msum_ps[64:65, 0:512], ones_bf[:],
                         zeros512[:], start=True, stop=True)
        nc.vector.tensor_tensor(
            out=O_bf[:].rearrange("p (c k) -> p c k", k=NCLS),
            in0=iota_sb[:].rearrange("p (a k) -> p a k", a=1)
            .to_broadcast((128, NCHUNK, NCLS)),
            in1=labels_sb[:].to_broadcast((128, NCHUNK, NCLS)),
            op=Alu.is_equal,
        )

        with (
            tc.tile_pool(name="main_ps", bufs=2, space="PSUM") as main_ps,
            tc.tile_pool(name="build", bufs=4) as build_pool,
            tc.tile_pool(name="esb", bufs=3) as esb_pool,
        ):
            # warm the PE's HAM clock gate while builds run (zero matmuls)
            warm_ps = main_ps.tile([128, 1024], f32, tag="e", name="warm_ps")
            for _ in range(24):
                nc.tensor.matmul(warm_ps[:, 0:512], zeros512[:, 0:128],
                                 zeros512[:], start=True, stop=True)

            # ---------------- builds (normalize + transpose + Msum) --------
            xh_halves = {}
            xs_tiles = {}

            def emit_dma(h):
                base = h * HALF
                xs = build_pool.tile([128, HALF], f32, tag="xs",
                                     name=f"xs{h}")
                nc.sync.dma_start(
                    xs[:].rearrange("p (c d) -> p c d", d=128),
                    x_dram[base:base + HALF, :].rearrange(
                        "(c p) d -> p c d", p=128),
                )
                xs_tiles[h] = xs

            def emit_build(h):
                base = h * HALF
                xs = xs_tiles.pop(h)
                sq = build_pool.tile([128, HALF], f32, tag="sq")
                nc.gpsimd.tensor_mul(sq[:], xs[:], xs[:])
                ssq = build_pool.tile([128, CH], f32, tag="ssq")
                nc.vector.reduce_sum(
                    ssq[:], sq[:].rearrange("p (c d) -> p c d", d=128), axis=X)
                lns = build_pool.tile([128, CH], f32, tag="lns")
                nc.scalar.activation(lns[:], ssq[:], Act.Ln)
                rinv = build_pool.tile([128, CH], f32, tag="rinv")
                nc.scalar.activation(rinv[:], lns[:], Act.Exp, scale=-0.5)
                if h < NXT:
                    xh = build_pool.tile([128, HALF], bf16, tag="xh")
                else:
                    xh = xh57[:, (h - NXT) * HALF:(h - NXT + 1) * HALF]
                nc.vector.scalar_tensor_tensor(
                    out=xh.rearrange("p (c r) -> p c r", r=128),
                    in0=xs[:].rearrange("p (c r) -> p c r", r=128),
                    scalar=1.0,
                    in1=rinv[:].to_broadcast((128, CH, 128)),
                    op0=Alu.mult,
                    op1=Alu.mult,
                )
                if h < NXT:
                    nc.sync.dma_start_transpose(
                        xT[:, base:base + HALF].rearrange(
                            "p (c r) -> p c r", r=128),
                        xh,
                    )
                if h + 4 < NHALF:
                    emit_dma(h + 4)
                if h == 0:
                    sq2 = build_pool.tile([128, HALF], f32, tag="sq2")
                    nc.vector.tensor_mul(sq2[:], xh, xh)
                    nc.vector.reduce_sum(
                        rawdiag[:],
                        sq2[:].rearrange("p (c d) -> p c d", d=128), axis=X)
                xh_halves[h] = xh

            def emit_msum(h):
                xh = xh_halves.pop(h)
                for i in range(CH):
                    c = h * CH + i
                    nc.tensor.matmul(
                        msum_ps[0:NCLS, 0:128],
                        O_bf[:, c * NCLS:(c + 1) * NCLS],
                        xh[:, i * 128:(i + 1) * 128],
                        start=(c == 0),
                        stop=(c == NCHUNK - 1),
                        skip_group_check=True,
                    )

            for h in range(4):
                emit_dma(h)
            for h in range(NXT):
                emit_build(h)
                emit_msum(h)

            # ---------------- main loop: band logits + exp + colsums -------
            prev_esb = None

            def emit_colsum(m, esb):
                for t, outc, eoff, w in _colsum_table(m):
                    bank, row = cs_slot(t)
                    nc.tensor.matmul(
                        bank[row:row + 1, outc:outc + w],
                        ones_bf[:],
                        esb[:, eoff:eoff + w],
                        start=False, stop=True,
                        skip_group_check=True,
                    )

            for m in range(NOWN):
                lhsT = xT[:, m * 128:(m + 1) * 128]
                esb = esb_pool.tile([128, MAINW], bf16, tag="esb")
                for kp in range(NPIECE):
                    off = kp * 1024
                    ps = main_ps.tile([128, 1024], f32, tag="e")
                    for k in range(2):
                        nc.tensor.matmul(
                            ps[:, k * 512:(k + 1) * 512],
                            lhsT,
                            xT[:, 128 * m + off + k * 512:
                               128 * m + off + (k + 1) * 512],
                            start=True, stop=True,
                        )
                    nc.scalar.activation(
                        esb[:, off:off + 1024], ps[:], Act.Exp,
                        scale=INV_TAU,
                        accum_out=Zacc[:, NPIECE * m + kp:NPIECE * m + kp + 1],
                    )
                if m > 0:
                    emit_colsum(m - 1, prev_esb)
                if m >= 2 and m <= 4:
                    h = m + 3          # builds 5..7 during main chunks 2..4
                    emit_build(h)
                    emit_msum(h)
                if m == 5:
                    # d32 blocks mid-loop: logits, exp (x0.5 via bias)
                    d32_ps = main_ps.tile([128, NOWN * 128], f32, tag="e",
                                          name="d32_ps")
                    for mm in range(NOWN):
                        nc.tensor.matmul(
                            d32_ps[:, 128 * mm:128 * mm + 128],
                            xT[:, mm * 128:(mm + 1) * 128],
                            xT[:, 128 * mm + MAINW:128 * mm + BANDW],
                            start=True, stop=True,
                        )
                    nc.scalar.activation(
                        e_d32[:], d32_ps[:], Act.Exp, scale=INV_TAU,
                        bias=lnhalf_sb[:])
                if m == 6:
                    nc.vector.reduce_sum(
                        Zd32r[:],
                        e_d32[:].rearrange("p (mm r) -> p mm r", r=128), axis=X)
                    for mm in range(NOWN):
                        g = 128 * mm + MAINW
                        bank, row = cs_slot(g // 512)
                        outc = g - 512 * (g // 512)
                        nc.tensor.matmul(
                            bank[row:row + 1, outc:outc + 128],
                            ones_bf[:],
                            e_d32[:, 128 * mm:128 * mm + 128],
                            start=False, stop=True,
                            skip_group_check=True,
                        )
                prev_esb = esb
            emit_colsum(NOWN - 1, prev_esb)

        # ---------------- tail / epilogue ----------------
        with tc.tile_pool(name="epi_ps", bufs=1, space="PSUM") as epi_ps:
            for b in range(3):
                nc.vector.tensor_copy(
                    zcol_sb[:, 512 * b:512 * (b + 1)], cs_banks[b][:])
            nc.vector.tensor_copy(zcol_sb[:, 1536:2048], msum_ps[:, 0:512])

            # Z row partials: main accums + d32 (self-term subtracted on host)
            nc.vector.reduce_sum(
                Zsum[:], Zacc[:].rearrange("p (m k) -> p m k", k=NPIECE), axis=X)
            nc.vector.tensor_add(zrow_sb[:], Zsum[:], Zd32r[:])

            # F = x_own @ Msum^T (host selects own class, computes S/P)
            nc.vector.tensor_copy(Msum_sb[:], msum_ps[0:NCLS, 0:128])
            smalls = epi_ps.tile([128, 512], f32, tag="smalls")
            mt_ps = smalls[:, 128:128 + NCLS]
            nc.tensor.transpose(mt_ps, Msum_sb[:], eye33_sb[:])
            nc.vector.tensor_copy(Mt_sb[:], mt_ps)
            F_ps = epi_ps.tile([128, NOWN * NCLS], f32, tag="F")
            for m in range(NOWN):
                nc.tensor.matmul(
                    F_ps[:, m * NCLS:(m + 1) * NCLS],
                    xT[:, m * 128:(m + 1) * 128],
                    Mt_sb[:],
                    start=True, stop=True,
                )
            nc.vector.tensor_copy(F_sb[:], F_ps[:])

            nc.sync.dma_start(zrow_dram[:], zrow_sb[:])
            nc.sync.dma_start(zcol_dram[:], zcol_sb[:])
            nc.sync.dma_start(f_dram[:], F_sb[:])
            nc.sync.dma_start(raw_dram[:], rawdiag[:])

    if split_waits:
        _split_multiwait(nc)
    return nc


def _get_nc(split_waits=True):
    global _NC
    if _NC is None:
        _NC = _build_nc(split_waits)
    return _NC


def _make_in_maps(x, lab):
    iota = np.ascontiguousarray(
        np.tile(np.arange(NCLS, dtype=np.float32), (128, 1))
    )
    in_maps = []
    for c in range(N_CORES):
        lo = c * ROWS_PER_CORE
        perm = np.concatenate([np.arange(lo, N), np.arange(0, lo)])
        xp = np.ascontiguousarray(x[perm])
        lp = np.ascontiguousarray(
            lab[perm].astype(np.float32).reshape(NCHUNK, 128).T
        )
        in_maps.append(
            {"xperm": xp, "labels_pc": lp, "iota33": iota,
             "eye33": np.eye(NCLS, dtype=np.float32)}
        )
    return in_maps


def _combine(results, lab):
    lab = np.asarray(lab).astype(np.int64)
    cnt = np.bincount(lab, minlength=NCLS)
    p128 = np.arange(128)
    l_loc = (128 * np.arange(NOWN)[None, :] + p128[:, None])   # [128, 8]
    Z = np.zeros(N, dtype=np.float64)
    raws = []
    for c in range(N_CORES):
        r = results[c]
        raw = np.asarray(r["raw"], dtype=np.float64)           # [128, 8]
        raws.append(raw)
        zrow = np.asarray(r["zrow"], dtype=np.float64)
        zrow_excl = zrow - np.exp(INV_TAU * raw)               # drop self term
        Zloc = np.zeros(N, dtype=np.float64)
        np.add.at(Zloc, l_loc.ravel(), zrow_excl.ravel())
        zc = np.asarray(r["zcol"], dtype=np.float64)
        # pieces 0-8: [32*(t%3), 512*(t//3)+c]; piece 9: [64, 1536+c]
        for t in range(10):
            if t == 9:
                colsum = zc[64, 1536:2048]
            else:
                colsum = zc[32 * (t % 3), 512 * (t // 3):512 * (t // 3) + 512]
            lo = max(128, 512 * t)
            hi = min(XTW, 512 * (t + 1))
            Zloc[lo:hi] += colsum[lo - 512 * t:hi - 512 * t]
        Z += np.roll(Zloc, ROWS_PER_CORE * c)
    loss_num = 0.0
    nvalid = 0.0
    for c in range(N_CORES):
        r = results[c]
        g = (ROWS_PER_CORE * c + l_loc) % N                    # [128, 8]
        labg = lab[g]                                          # [128, 8]
        F = np.asarray(r["fcls"], dtype=np.float64).reshape(128, NOWN, NCLS)
        S_full = np.take_along_axis(
            F, labg[:, :, None], axis=2)[:, :, 0]              # [128, 8]
        S_excl = S_full - raws[c]
        P = cnt[labg] - 1
        val = (P > 0).astype(np.float64)
        tsp = INV_TAU * S_excl / np.maximum(P, 1)
        lnZ = np.log(Z[g])
        loss_num += ((tsp - lnZ) * val).sum()
        nvalid += val.sum()
    return np.array(-loss_num / nvalid, dtype=np.float32)


def kernel(feature_embeds, label_ids):
    from concourse.bass_utils import run_bass_kernel_spmd

    x = np.asarray(feature_embeds, dtype=np.float32)
    lab = np.asarray(label_ids)
    nc = _get_nc()
    res = run_bass_kernel_spmd(nc, _make_in_maps(x, lab), list(range(N_CORES)))
    return _combine(res.results, lab)


def kernel_profiled(feature_embeds, label_ids):
    """Same as kernel(), but with NTFF tracing; returns (loss, exec_time_ns)."""
    print("ntff hook installed:", _install_ntff_hook())
    from concourse.bass_utils import run_bass_kernel_spmd

    x = np.asarray(feature_embeds, dtype=np.float32)
    lab = np.asarray(label_ids)
    nc = _get_nc()
    res = run_bass_kernel_spmd(
        nc, _make_in_maps(x, lab), list(range(N_CORES)), trace=True
    )
    return _combine(res.results, lab), res.exec_time_ns
